# revision 19
# baseline (speedup 1.0000x reference)
"""Bahdanau attention on 8 Trainium2 NeuronCores.

Problem (full shapes): hidden [32,1024], encoder_outputs [2048,32,2048],
Wa [1024,1024], Ua [1024,2048], Va [1,1024].

reference:
    enc    = encoder_outputs.transpose(1,0,2)        # [B,S,2H]
    h_proj = hidden @ Wa.T                           # [B,H]
    e_proj = einsum('bsd,hd->bsh', enc, Ua)          # [B,S,H]
    energy = relu(h_proj[:,None,:] + e_proj)         # [B,S,H]
    scores = einsum('bsh,h->bs', energy, Va[0])      # [B,S]
    attn   = softmax(scores, axis=-1)                # [B,S]
    ctx    = einsum('bs,bsd->bd', attn, enc)[:,None] # [B,1,2H]
    return (ctx, attn)

Strategy: data-parallel over batch (4 batches/core).  Per batch the
dominant matmul is enc_b @ Ua.T, computed transposed as e_projT[h,s] so
the relu+h_proj bias is a per-partition scalar on the scalar engine and
the Va contraction is a K=H matmul on the tensor engine.  Softmax is
computed unnormalized (scores are O(+-10) so exp needs no max shift).
The context vector is accumulated flash-style per s-chunk directly from
the encoder tiles already resident in SBUF: multiply by the exp-weight
row (broadcast across partitions) on GpSimd, reduce along s on VectorE,
and rescale by 1/Z at the end.  encoder_outputs is therefore read from
HBM exactly once.  Host pre-transposes it so the e_proj contraction dim
lands on SBUF partitions.
"""

import os
import sys

sys.path.insert(0, "/opt/trn_rl_repo")

import numpy as np
import ml_dtypes

import concourse.bacc as bacc
import concourse.tile as tile
import concourse.mybir as mybir
from concourse import bass_utils

B, S, H = 32, 2048, 1024
D = 2 * H
NCORES = 8
BL = B // NCORES  # batches per core
P = 128
NCH = 512          # matmul moving free-dim chunk (one PSUM bank)
KD = D // P        # 16 contraction tiles over D (e_proj)
KH = H // P        # 8 contraction tiles over H (scores)
NS = S // NCH      # 4 s-chunks

MM_DT = os.environ.get("BAH_MM_DT", "bf16")

_CACHE = {}


def _build(mm_dt: str):
    assert mm_dt == "bf16"
    f32 = mybir.dt.float32
    mdt = mybir.dt.bfloat16

    nc = bacc.Bacc("TRN2", target_bir_lowering=False, debug=False)
    encT = nc.declare_dram_parameter("encT", [BL, D, S], f32, isOutput=False)
    uaT = nc.declare_dram_parameter("uaT", [D, H], mdt, isOutput=False)
    waT = nc.declare_dram_parameter("waT", [H, H], f32, isOutput=False)
    hidT = nc.declare_dram_parameter("hidT", [H, BL], f32, isOutput=False)
    va2 = nc.declare_dram_parameter("va2", [P, KH], mdt, isOutput=False)
    id128 = nc.declare_dram_parameter("id128", [P, P], mdt, isOutput=False)
    out_ctx = nc.declare_dram_parameter("out_ctx", [BL, D], f32, isOutput=True)
    out_attn = nc.declare_dram_parameter("out_attn", [BL, S], f32, isOutput=True)

    AF = mybir.ActivationFunctionType

    with tile.TileContext(nc) as tc:
        with (
            tc.tile_pool(name="weights", bufs=1) as wpool,
            tc.tile_pool(name="wa", bufs=6) as wapool,
            tc.tile_pool(name="et", bufs=3) as etpool,
            tc.tile_pool(name="etm", bufs=4) as etmpool,
            tc.tile_pool(name="en", bufs=3) as enpool,
            tc.tile_pool(name="sm", bufs=2) as smpool,
            tc.tile_pool(name="misc", bufs=2) as miscpool,
            tc.tile_pool(name="pe", bufs=3, space="PSUM") as pe_psum,
            tc.tile_pool(name="small", bufs=3, space="PSUM") as small_psum,
        ):
            # ---- persistent weights (ua split per k-tile so the first
            # matmul only waits for one 256KB slice) ----
            ua_sb = wpool.tile([P, KD, H], mdt)
            uaT_r = uaT[:].rearrange("(ko p) h -> p ko h", p=P)
            for k in range(KD):
                nc.sync.dma_start(ua_sb[:, k : k + 1, :], uaT_r[:, k : k + 1, :])
            va_sb = wpool.tile([P, KH], mdt)
            nc.sync.dma_start(va_sb[:], va2[:])
            id_sb = wpool.tile([P, P], mdt)
            nc.sync.dma_start(id_sb[:], id128[:])
            hid_sb = wpool.tile([P, KH, BL], f32)
            nc.sync.dma_start(hid_sb[:], hidT[:].rearrange("(ko p) b -> p ko b", p=P))
            hp_sb = wpool.tile([P, KH, BL], f32)

            def emit_hp(m):
                # hp[h, b] = sum_k Wa.T[k, h] * hidden.T[k, b] for h-tile m
                php = small_psum.tile([P, BL], f32, tag="sp", name="php")
                for k in range(KH):
                    wa_t = wapool.tile([P, P], f32, tag="wa")
                    nc.sync.dma_start(
                        wa_t[:], waT[k * P : (k + 1) * P, m * P : (m + 1) * P]
                    )
                    nc.tensor.matmul(
                        php[:],
                        lhsT=wa_t[:],
                        rhs=hid_sb[:, k, :],
                        start=(k == 0),
                        stop=(k == KH - 1),
                    )
                nc.vector.tensor_copy(hp_sb[:, m, :], php[:])

            pending_finish = None
            for b in range(BL):
                scores = smpool.tile([1, S], f32, tag="scores")
                zrow = miscpool.tile([1, NS], f32, tag="zrow")
                acc = miscpool.tile([P, KD], f32, tag="acc")
                for n in range(NS):
                    ssl = slice(n * NCH, (n + 1) * NCH)
                    ets = []
                    for h in range(2):
                        et = etpool.tile([P, KD // 2, NCH], f32, tag="et")
                        nc.sync.dma_start(
                            et[:],
                            encT[b, h * (D // 2) : (h + 1) * (D // 2), ssl].rearrange(
                                "(ko p) s -> p ko s", p=P
                            ),
                        )
                        ets.append(et)
                    etm = etmpool.tile([P, KD, NCH], mdt, tag="etm")
                    nc.vector.tensor_copy(etm[:, 0 : KD // 2, :], ets[0][:])
                    nc.vector.tensor_copy(etm[:, KD // 2 : KD, :], ets[1][:])

                    scp = small_psum.tile([1, NCH], f32, tag="sp", name="scp")
                    for m in range(KH):
                        if b == 0 and n == 0:
                            emit_hp(m)
                        ps = pe_psum.tile([P, NCH], f32, tag="pe")
                        for k in range(KD):
                            nc.tensor.matmul(
                                ps[:],
                                lhsT=ua_sb[:, k, m * P : (m + 1) * P],
                                rhs=etm[:, k, :],
                                start=(k == 0),
                                stop=(k == KD - 1),
                            )
                        en = enpool.tile([P, NCH], mdt, tag="en")
                        nc.scalar.activation(
                            en[:], ps[:], AF.Relu, bias=hp_sb[:, m, b : b + 1]
                        )
                        nc.tensor.matmul(
                            scp[:],
                            lhsT=va_sb[:, m : m + 1],
                            rhs=en[:],
                            start=(m == 0),
                            stop=(m == KH - 1),
                        )
                    # exp (unnormalized softmax numerator) + running Z
                    nc.vector.tensor_copy(scores[:, ssl], scp[:])
                    nc.scalar.activation(
                        scores[:, ssl],
                        scores[:, ssl],
                        AF.Exp,
                        accum_out=zrow[:, n : n + 1],
                    )
                    exp_m = smpool.tile([1, NCH], mdt, tag="expm")
                    nc.vector.tensor_copy(exp_m[:], scores[:, ssl])
                    # broadcast exp row to all partitions, then fold the
                    # context contribution of this s-chunk out of the very
                    # encoder tiles the e_proj matmul just consumed:
                    # acc[p, k] += sum_s etm[p, k, s] * exp[s]
                    w_bc = miscpool.tile([P, NCH], mdt, tag="wbc")
                    nc.gpsimd.partition_broadcast(w_bc[:], exp_m[:])
                    nc.gpsimd.tensor_tensor(
                        etm[:],
                        etm[:],
                        w_bc[:, None, :].to_broadcast((P, KD, NCH)),
                        mybir.AluOpType.mult,
                    )
                    if n == 0:
                        nc.vector.reduce_sum(
                            acc[:], etm[:], axis=mybir.AxisListType.X
                        )
                    else:
                        rtmp = miscpool.tile([P, KD], f32, tag="rtmp")
                        nc.vector.reduce_sum(
                            rtmp[:], etm[:], axis=mybir.AxisListType.X
                        )
                        nc.vector.tensor_add(acc[:], acc[:], rtmp[:])
                    # emit the previous batch's normalize+output block here,
                    # after this batch's first chunk has filled the PE queue,
                    # so the PE never idles waiting on the softmax chain
                    if n == 0 and pending_finish is not None:
                        pending_finish()
                        pending_finish = None

                def finish(b=b, scores=scores, zrow=zrow, acc=acc):
                    # ---- normalize: Z, attn out, ctx out ----
                    zsum = miscpool.tile([1, 1], f32, tag="zsum")
                    nc.vector.reduce_sum(zsum[:], zrow[:], axis=mybir.AxisListType.X)
                    sinv = miscpool.tile([1, 1], f32, tag="sinv")
                    nc.vector.reciprocal(sinv[:], zsum[:])
                    nc.vector.tensor_scalar_mul(scores[:], scores[:], sinv[:])
                    nc.sync.dma_start(out_attn[b : b + 1, :], scores[:])
                    sinv_bc = miscpool.tile([P, 1], f32, tag="sinvbc")
                    nc.gpsimd.partition_broadcast(sinv_bc[:], sinv[:])
                    acc_bf = miscpool.tile([P, KD], mdt, tag="accbf")
                    nc.vector.tensor_scalar_mul(acc_bf[:], acc[:], sinv_bc[:])
                    ptx = small_psum.tile([KD, P], f32, tag="sp", name="ptx")
                    nc.tensor.matmul(
                        ptx[:], lhsT=acc_bf[:], rhs=id_sb[:], start=True, stop=True
                    )
                    ctxrow = miscpool.tile([KD, P], f32, tag="ctxrow")
                    nc.vector.tensor_copy(ctxrow[:], ptx[:])
                    nc.sync.dma_start(
                        out_ctx[b].rearrange("(ko p) -> ko p", p=P), ctxrow[:]
                    )

                pending_finish = finish
            pending_finish()

    nc.compile()
    return nc


def _get_nc(mm_dt: str):
    if mm_dt not in _CACHE:
        _CACHE[mm_dt] = _build(mm_dt)
    return _CACHE[mm_dt]


def kernel(hidden, encoder_outputs, Wa, Ua, Va, _trace=False):
    mm_dt = MM_DT
    nc = _get_nc(mm_dt)

    wdt = ml_dtypes.bfloat16

    # host-side layout prep (sharding)
    encT_all = np.ascontiguousarray(encoder_outputs.transpose(1, 2, 0))  # [B, D, S]
    uaT_np = np.ascontiguousarray(Ua.T).astype(wdt)  # [D, H]
    waT_np = np.ascontiguousarray(Wa.T).astype(np.float32)  # [H, H]
    va2_np = np.ascontiguousarray(Va[0].reshape(KH, P).T).astype(wdt)  # [P, KH]
    id_np = np.eye(P, dtype=wdt)

    in_maps = []
    for c in range(NCORES):
        b0 = c * BL
        in_maps.append(
            {
                "encT": encT_all[b0 : b0 + BL],
                "uaT": uaT_np,
                "waT": waT_np,
                "hidT": np.ascontiguousarray(hidden[b0 : b0 + BL].T),
                "va2": va2_np,
                "id128": id_np,
            }
        )

    res = bass_utils.run_bass_kernel_spmd(
        nc, in_maps, core_ids=list(range(NCORES)), trace=_trace
    )

    ctx = np.concatenate([res.results[c]["out_ctx"] for c in range(NCORES)], axis=0)
    attn = np.concatenate([res.results[c]["out_attn"] for c in range(NCORES)], axis=0)
    out = (ctx.reshape(B, 1, D).astype(np.float32), attn.astype(np.float32))
    if _trace:
        return out, res
    return out


# revision 20
# speedup vs baseline: 1.2912x; 1.2912x over previous
"""Bahdanau attention on 8 Trainium2 NeuronCores.

Problem (full shapes): hidden [32,1024], encoder_outputs [2048,32,2048],
Wa [1024,1024], Ua [1024,2048], Va [1,1024].

reference:
    enc    = encoder_outputs.transpose(1,0,2)        # [B,S,2H]
    h_proj = hidden @ Wa.T                           # [B,H]
    e_proj = einsum('bsd,hd->bsh', enc, Ua)          # [B,S,H]
    energy = relu(h_proj[:,None,:] + e_proj)         # [B,S,H]
    scores = einsum('bsh,h->bs', energy, Va[0])      # [B,S]
    attn   = softmax(scores, axis=-1)                # [B,S]
    ctx    = einsum('bs,bsd->bd', attn, enc)[:,None] # [B,1,2H]
    return (ctx, attn)

Strategy: data-parallel over batch (4 batches/core).  Per batch the
dominant matmul is enc_b @ Ua.T, computed transposed as e_projT[h,s] so
the relu+h_proj bias is a per-partition scalar on the scalar engine and
the Va contraction is a K=H matmul on the tensor engine.  Softmax is
computed unnormalized (scores are O(+-10) so exp needs no max shift).
The context vector is folded flash-style per s-chunk out of the very
encoder tiles the e_proj matmul consumes, with fused multiply-reduce
(tensor_tensor_reduce) ops on VectorE against the exp-weight row
(partition-broadcast), rescaling by 1/Z at the end.  encoder_outputs is
therefore read from HBM exactly once.  Chunk loads/casts are emitted
one chunk ahead of the reduce chain so the in-order VectorE queue never
delays the tensor engine, and each batch's epilogue is deferred into
the next batch's instruction stream.
"""

import os
import sys

sys.path.insert(0, "/opt/trn_rl_repo")

import numpy as np
import ml_dtypes

import concourse.bacc as bacc
import concourse.tile as tile
import concourse.mybir as mybir
from concourse import bass_utils

B, S, H = 32, 2048, 1024
D = 2 * H
NCORES = 8
BL = B // NCORES  # batches per core
P = 128
NCH = 512          # matmul moving free-dim chunk (one PSUM bank)
KD = D // P        # 16 contraction tiles over D (e_proj)
KH = H // P        # 8 contraction tiles over H (scores)
NS = S // NCH      # 4 s-chunks

MM_DT = os.environ.get("BAH_MM_DT", "bf16")

_CACHE = {}


def _build(mm_dt: str):
    assert mm_dt == "bf16"
    f32 = mybir.dt.float32
    mdt = mybir.dt.bfloat16

    nc = bacc.Bacc("TRN2", target_bir_lowering=False, debug=False)
    encT = nc.declare_dram_parameter("encT", [BL, D, S], f32, isOutput=False)
    uaT = nc.declare_dram_parameter("uaT", [D, H], mdt, isOutput=False)
    waT = nc.declare_dram_parameter("waT", [H, H], f32, isOutput=False)
    hidT = nc.declare_dram_parameter("hidT", [H, BL], f32, isOutput=False)
    va2 = nc.declare_dram_parameter("va2", [P, KH], mdt, isOutput=False)
    id128 = nc.declare_dram_parameter("id128", [P, P], mdt, isOutput=False)
    out_ctx = nc.declare_dram_parameter("out_ctx", [BL, D], f32, isOutput=True)
    out_attn = nc.declare_dram_parameter("out_attn", [BL, S], f32, isOutput=True)

    AF = mybir.ActivationFunctionType

    with tile.TileContext(nc) as tc:
        with (
            tc.tile_pool(name="weights", bufs=1) as wpool,
            tc.tile_pool(name="wa", bufs=6) as wapool,
            tc.tile_pool(name="et", bufs=3) as etpool,
            tc.tile_pool(name="etm", bufs=4) as etmpool,
            tc.tile_pool(name="en", bufs=3) as enpool,
            tc.tile_pool(name="sm", bufs=2) as smpool,
            tc.tile_pool(name="misc", bufs=2) as miscpool,
            tc.tile_pool(name="pe", bufs=3, space="PSUM") as pe_psum,
            tc.tile_pool(name="small", bufs=3, space="PSUM") as small_psum,
        ):
            def load_chunk(b, n):
                # DMA one [D, NCH] slab of encT (two halves) and cast to bf16
                ssl = slice(n * NCH, (n + 1) * NCH)
                etm = etmpool.tile([P, KD, NCH], mdt, tag="etm", name="etm")
                for h in range(2):
                    et = etpool.tile([P, KD // 2, NCH], f32, tag="et", name="et")
                    nc.sync.dma_start(
                        et[:],
                        encT[b, h * (D // 2) : (h + 1) * (D // 2), ssl].rearrange(
                            "(ko p) s -> p ko s", p=P
                        ),
                    )
                    nc.vector.tensor_copy(
                        etm[:, h * (KD // 2) : (h + 1) * (KD // 2), :], et[:]
                    )
                return etm

            # first chunk's loads go out before the weight DMAs so the
            # tensor engine can start as early as possible
            etm_cur = load_chunk(0, 0)

            ua_sb = wpool.tile([P, KD, H], mdt)
            uaT_r = uaT[:].rearrange("(ko p) h -> p ko h", p=P)
            for k in range(KD):
                nc.sync.dma_start(ua_sb[:, k : k + 1, :], uaT_r[:, k : k + 1, :])
            va_sb = wpool.tile([P, KH], mdt)
            nc.sync.dma_start(va_sb[:], va2[:])
            id_sb = wpool.tile([P, P], mdt)
            nc.sync.dma_start(id_sb[:], id128[:])
            hid_sb = wpool.tile([P, KH, BL], f32)
            nc.sync.dma_start(hid_sb[:], hidT[:].rearrange("(ko p) b -> p ko b", p=P))
            hp_sb = wpool.tile([P, KH, BL], f32)

            def emit_hp(m):
                # hp[h, b] = sum_k Wa.T[k, h] * hidden.T[k, b] for h-tile m
                php = small_psum.tile([P, BL], f32, tag="sp", name="php")
                for k in range(KH):
                    wa_t = wapool.tile([P, P], f32, tag="wa")
                    nc.sync.dma_start(
                        wa_t[:], waT[k * P : (k + 1) * P, m * P : (m + 1) * P]
                    )
                    nc.tensor.matmul(
                        php[:],
                        lhsT=wa_t[:],
                        rhs=hid_sb[:, k, :],
                        start=(k == 0),
                        stop=(k == KH - 1),
                    )
                nc.vector.tensor_copy(hp_sb[:, m, :], php[:])

            pending_finish = None
            for b in range(BL):
                scores = smpool.tile([1, S], f32, tag="scores")
                zrow = miscpool.tile([1, NS], f32, tag="zrow")
                acc = miscpool.tile([P, KD], f32, tag="acc")
                for n in range(NS):
                    ssl = slice(n * NCH, (n + 1) * NCH)
                    etm = etm_cur
                    # prefetch the next chunk ahead of this chunk's reduce
                    # ops so the VectorE FIFO can't stall the tensor engine
                    if n + 1 < NS:
                        etm_cur = load_chunk(b, n + 1)
                    elif b + 1 < BL:
                        etm_cur = load_chunk(b + 1, 0)

                    scp = small_psum.tile([1, NCH], f32, tag="sp", name="scp")
                    for m in range(KH):
                        if b == 0 and n == 0:
                            emit_hp(m)
                        ps = pe_psum.tile([P, NCH], f32, tag="pe")
                        for k in range(KD):
                            nc.tensor.matmul(
                                ps[:],
                                lhsT=ua_sb[:, k, m * P : (m + 1) * P],
                                rhs=etm[:, k, :],
                                start=(k == 0),
                                stop=(k == KD - 1),
                            )
                        en = enpool.tile([P, NCH], mdt, tag="en")
                        nc.scalar.activation(
                            en[:], ps[:], AF.Relu, bias=hp_sb[:, m, b : b + 1]
                        )
                        nc.tensor.matmul(
                            scp[:],
                            lhsT=va_sb[:, m : m + 1],
                            rhs=en[:],
                            start=(m == 0),
                            stop=(m == KH - 1),
                        )
                    # exp (unnormalized softmax numerator) + running Z
                    nc.vector.tensor_copy(scores[:, ssl], scp[:])
                    nc.scalar.activation(
                        scores[:, ssl],
                        scores[:, ssl],
                        AF.Exp,
                        accum_out=zrow[:, n : n + 1],
                    )
                    exp_m = smpool.tile([1, NCH], mdt, tag="expm")
                    nc.vector.tensor_copy(exp_m[:], scores[:, ssl])
                    w_bc = miscpool.tile([P, NCH], mdt, tag="wbc")
                    nc.gpsimd.partition_broadcast(w_bc[:], exp_m[:])
                    # fused multiply+reduce folds this s-chunk's context
                    # contribution out of the encoder tiles in SBUF:
                    # acc[p, k] += sum_s etm[p, k, s] * exp[s]
                    rtmp = miscpool.tile([P, KD], f32, tag="rtmp", name="rtmp")
                    for k in range(KD):
                        junk = miscpool.tile([P, NCH], mdt, tag="junk", name="junk")
                        nc.vector.scalar_tensor_tensor(
                            out=junk[:],
                            in0=etm[:, k, :],
                            scalar=1.0,
                            in1=w_bc[:],
                            op0=mybir.AluOpType.mult,
                            op1=mybir.AluOpType.mult,
                            accum_out=rtmp[:, k : k + 1],
                        )
                    if n == 0:
                        nc.vector.tensor_copy(acc[:], rtmp[:])
                    else:
                        nc.vector.tensor_add(acc[:], acc[:], rtmp[:])
                    # emit the previous batch's normalize+output block here,
                    # after this batch's first chunk has filled the PE queue
                    if n == 0 and pending_finish is not None:
                        pending_finish()
                        pending_finish = None

                def finish(b=b, scores=scores, zrow=zrow, acc=acc):
                    # ---- normalize: Z, attn out, ctx out ----
                    zsum = miscpool.tile([1, 1], f32, tag="zsum")
                    nc.vector.reduce_sum(zsum[:], zrow[:], axis=mybir.AxisListType.X)
                    sinv = miscpool.tile([1, 1], f32, tag="sinv")
                    nc.vector.reciprocal(sinv[:], zsum[:])
                    nc.vector.tensor_scalar_mul(scores[:], scores[:], sinv[:])
                    nc.sync.dma_start(out_attn[b : b + 1, :], scores[:])
                    sinv_bc = miscpool.tile([P, 1], f32, tag="sinvbc")
                    nc.gpsimd.partition_broadcast(sinv_bc[:], sinv[:])
                    acc_bf = miscpool.tile([P, KD], mdt, tag="accbf")
                    nc.vector.tensor_scalar_mul(acc_bf[:], acc[:], sinv_bc[:])
                    ptx = small_psum.tile([KD, P], f32, tag="sp", name="ptx")
                    nc.tensor.matmul(
                        ptx[:], lhsT=acc_bf[:], rhs=id_sb[:], start=True, stop=True
                    )
                    ctxrow = miscpool.tile([KD, P], f32, tag="ctxrow")
                    nc.vector.tensor_copy(ctxrow[:], ptx[:])
                    nc.sync.dma_start(
                        out_ctx[b].rearrange("(ko p) -> ko p", p=P), ctxrow[:]
                    )

                pending_finish = finish
            pending_finish()

    nc.compile()
    return nc


def _get_nc(mm_dt: str):
    if mm_dt not in _CACHE:
        _CACHE[mm_dt] = _build(mm_dt)
    return _CACHE[mm_dt]


def kernel(hidden, encoder_outputs, Wa, Ua, Va, _trace=False):
    mm_dt = MM_DT
    nc = _get_nc(mm_dt)

    wdt = ml_dtypes.bfloat16

    # host-side layout prep (sharding)
    encT_all = np.ascontiguousarray(encoder_outputs.transpose(1, 2, 0))  # [B, D, S]
    uaT_np = np.ascontiguousarray(Ua.T).astype(wdt)  # [D, H]
    waT_np = np.ascontiguousarray(Wa.T).astype(np.float32)  # [H, H]
    va2_np = np.ascontiguousarray(Va[0].reshape(KH, P).T).astype(wdt)  # [P, KH]
    id_np = np.eye(P, dtype=wdt)

    in_maps = []
    for c in range(NCORES):
        b0 = c * BL
        in_maps.append(
            {
                "encT": encT_all[b0 : b0 + BL],
                "uaT": uaT_np,
                "waT": waT_np,
                "hidT": np.ascontiguousarray(hidden[b0 : b0 + BL].T),
                "va2": va2_np,
                "id128": id_np,
            }
        )

    res = bass_utils.run_bass_kernel_spmd(
        nc, in_maps, core_ids=list(range(NCORES)), trace=_trace
    )

    ctx = np.concatenate([res.results[c]["out_ctx"] for c in range(NCORES)], axis=0)
    attn = np.concatenate([res.results[c]["out_attn"] for c in range(NCORES)], axis=0)
    out = (ctx.reshape(B, 1, D).astype(np.float32), attn.astype(np.float32))
    if _trace:
        return out, res
    return out


# revision 21
# speedup vs baseline: 1.2950x; 1.0030x over previous
"""Bahdanau attention on 8 Trainium2 NeuronCores.

Problem (full shapes): hidden [32,1024], encoder_outputs [2048,32,2048],
Wa [1024,1024], Ua [1024,2048], Va [1,1024].

reference:
    enc    = encoder_outputs.transpose(1,0,2)        # [B,S,2H]
    h_proj = hidden @ Wa.T                           # [B,H]
    e_proj = einsum('bsd,hd->bsh', enc, Ua)          # [B,S,H]
    energy = relu(h_proj[:,None,:] + e_proj)         # [B,S,H]
    scores = einsum('bsh,h->bs', energy, Va[0])      # [B,S]
    attn   = softmax(scores, axis=-1)                # [B,S]
    ctx    = einsum('bs,bsd->bd', attn, enc)[:,None] # [B,1,2H]
    return (ctx, attn)

Strategy: data-parallel over batch (4 batches/core).  Per batch the
dominant matmul is enc_b @ Ua.T, computed transposed as e_projT[h,s] so
the relu+h_proj bias is a per-partition scalar on the scalar engine and
the Va contraction is a K=H matmul on the tensor engine.  Softmax is
computed unnormalized (scores are O(+-10) so exp needs no max shift).
The context vector is folded flash-style per s-chunk out of the very
encoder tiles the e_proj matmul consumes, with fused multiply-reduce
(tensor_tensor_reduce) ops on VectorE against the exp-weight row
(partition-broadcast), rescaling by 1/Z at the end.  encoder_outputs is
therefore read from HBM exactly once.  Chunk loads/casts are emitted
one chunk ahead of the reduce chain so the in-order VectorE queue never
delays the tensor engine, and each batch's epilogue is deferred into
the next batch's instruction stream.
"""

import os
import sys

sys.path.insert(0, "/opt/trn_rl_repo")

import numpy as np
import ml_dtypes

import concourse.bacc as bacc
import concourse.tile as tile
import concourse.mybir as mybir
from concourse import bass_utils

B, S, H = 32, 2048, 1024
D = 2 * H
NCORES = 8
BL = B // NCORES  # batches per core
P = 128
NCH = 512          # matmul moving free-dim chunk (one PSUM bank)
KD = D // P        # 16 contraction tiles over D (e_proj)
KH = H // P        # 8 contraction tiles over H (scores)
NS = S // NCH      # 4 s-chunks

MM_DT = os.environ.get("BAH_MM_DT", "bf16")

_CACHE = {}


def _build(mm_dt: str):
    assert mm_dt == "bf16"
    f32 = mybir.dt.float32
    mdt = mybir.dt.bfloat16

    nc = bacc.Bacc("TRN2", target_bir_lowering=False, debug=False)
    encT = nc.declare_dram_parameter("encT", [BL, D, S], f32, isOutput=False)
    uaT = nc.declare_dram_parameter("uaT", [D, H], mdt, isOutput=False)
    waT = nc.declare_dram_parameter("waT", [H, H], mdt, isOutput=False)
    hidT = nc.declare_dram_parameter("hidT", [H, BL], mdt, isOutput=False)
    va2 = nc.declare_dram_parameter("va2", [P, KH], mdt, isOutput=False)
    id128 = nc.declare_dram_parameter("id128", [P, P], mdt, isOutput=False)
    out_ctx = nc.declare_dram_parameter("out_ctx", [BL, D], f32, isOutput=True)
    out_attn = nc.declare_dram_parameter("out_attn", [BL, S], f32, isOutput=True)

    AF = mybir.ActivationFunctionType

    with tile.TileContext(nc) as tc:
        with (
            tc.tile_pool(name="weights", bufs=1) as wpool,
            tc.tile_pool(name="wa", bufs=6) as wapool,
            tc.tile_pool(name="et", bufs=3) as etpool,
            tc.tile_pool(name="etm", bufs=4) as etmpool,
            tc.tile_pool(name="en", bufs=3) as enpool,
            tc.tile_pool(name="sm", bufs=2) as smpool,
            tc.tile_pool(name="misc", bufs=2) as miscpool,
            tc.tile_pool(name="pe", bufs=3, space="PSUM") as pe_psum,
            tc.tile_pool(name="small", bufs=3, space="PSUM") as small_psum,
        ):
            def load_chunk(b, n, pieces=2):
                # DMA one [D, NCH] slab of encT and cast to bf16; more pieces
                # -> the first matmul can start sooner (used for chunk 0)
                ssl = slice(n * NCH, (n + 1) * NCH)
                etm = etmpool.tile([P, KD, NCH], mdt, tag="etm", name="etm")
                kq = KD // pieces
                for h in range(pieces):
                    et = etpool.tile(
                        [P, KD // 2, NCH], f32, tag="et", name="et"
                    )[:, :kq, :]
                    nc.sync.dma_start(
                        et[:],
                        encT[b, h * kq * P : (h + 1) * kq * P, ssl].rearrange(
                            "(ko p) s -> p ko s", p=P
                        ),
                    )
                    nc.vector.tensor_copy(etm[:, h * kq : (h + 1) * kq, :], et[:])
                return etm

            # first chunk's loads go out before the weight DMAs so the
            # tensor engine can start as early as possible
            etm_cur = load_chunk(0, 0, pieces=4)

            ua_sb = wpool.tile([P, KD, H], mdt)
            uaT_r = uaT[:].rearrange("(ko p) h -> p ko h", p=P)
            for k in range(KD):
                nc.sync.dma_start(ua_sb[:, k : k + 1, :], uaT_r[:, k : k + 1, :])
            va_sb = wpool.tile([P, KH], mdt)
            nc.sync.dma_start(va_sb[:], va2[:])
            id_sb = wpool.tile([P, P], mdt)
            nc.sync.dma_start(id_sb[:], id128[:])
            hid_sb = wpool.tile([P, KH, BL], mdt)
            nc.sync.dma_start(hid_sb[:], hidT[:].rearrange("(ko p) b -> p ko b", p=P))
            hp_sb = wpool.tile([P, KH, BL], f32)

            def emit_hp(m):
                # hp[h, b] = sum_k Wa.T[k, h] * hidden.T[k, b] for h-tile m
                php = small_psum.tile([P, BL], f32, tag="sp", name="php")
                for k in range(KH):
                    wa_t = wapool.tile([P, P], mdt, tag="wa")
                    nc.sync.dma_start(
                        wa_t[:], waT[k * P : (k + 1) * P, m * P : (m + 1) * P]
                    )
                    nc.tensor.matmul(
                        php[:],
                        lhsT=wa_t[:],
                        rhs=hid_sb[:, k, :],
                        start=(k == 0),
                        stop=(k == KH - 1),
                    )
                nc.vector.tensor_copy(hp_sb[:, m, :], php[:])

            pending_finish = None
            for b in range(BL):
                scores = smpool.tile([1, S], f32, tag="scores")
                zrow = miscpool.tile([1, NS], f32, tag="zrow")
                acc = miscpool.tile([P, KD], f32, tag="acc")
                for n in range(NS):
                    ssl = slice(n * NCH, (n + 1) * NCH)
                    etm = etm_cur
                    # prefetch the next chunk ahead of this chunk's reduce
                    # ops so the VectorE FIFO can't stall the tensor engine
                    if n + 1 < NS:
                        etm_cur = load_chunk(b, n + 1)
                    elif b + 1 < BL:
                        etm_cur = load_chunk(b + 1, 0)

                    scp = small_psum.tile([1, NCH], f32, tag="sp", name="scp")
                    for m in range(KH):
                        ps = pe_psum.tile([P, NCH], f32, tag="pe")
                        for k in range(KD):
                            nc.tensor.matmul(
                                ps[:],
                                lhsT=ua_sb[:, k, m * P : (m + 1) * P],
                                rhs=etm[:, k, :],
                                start=(k == 0),
                                stop=(k == KD - 1),
                            )
                        if b == 0 and n == 0:
                            emit_hp(m)
                        en = enpool.tile([P, NCH], mdt, tag="en")
                        nc.scalar.activation(
                            en[:], ps[:], AF.Relu, bias=hp_sb[:, m, b : b + 1]
                        )
                        nc.tensor.matmul(
                            scp[:],
                            lhsT=va_sb[:, m : m + 1],
                            rhs=en[:],
                            start=(m == 0),
                            stop=(m == KH - 1),
                        )
                    # exp (unnormalized softmax numerator) + running Z
                    nc.vector.tensor_copy(scores[:, ssl], scp[:])
                    nc.scalar.activation(
                        scores[:, ssl],
                        scores[:, ssl],
                        AF.Exp,
                        accum_out=zrow[:, n : n + 1],
                    )
                    exp_m = smpool.tile([1, NCH], mdt, tag="expm")
                    nc.vector.tensor_copy(exp_m[:], scores[:, ssl])
                    w_bc = miscpool.tile([P, NCH], mdt, tag="wbc")
                    nc.gpsimd.partition_broadcast(w_bc[:], exp_m[:])
                    # fused multiply+reduce folds this s-chunk's context
                    # contribution out of the encoder tiles in SBUF:
                    # acc[p, k] += sum_s etm[p, k, s] * exp[s]
                    rtmp = miscpool.tile([P, KD], f32, tag="rtmp", name="rtmp")
                    for k in range(KD):
                        junk = miscpool.tile([P, NCH], mdt, tag="junk", name="junk")
                        nc.vector.scalar_tensor_tensor(
                            out=junk[:],
                            in0=etm[:, k, :],
                            scalar=1.0,
                            in1=w_bc[:],
                            op0=mybir.AluOpType.mult,
                            op1=mybir.AluOpType.mult,
                            accum_out=rtmp[:, k : k + 1],
                        )
                    if n == 0:
                        nc.vector.tensor_copy(acc[:], rtmp[:])
                    else:
                        nc.vector.tensor_add(acc[:], acc[:], rtmp[:])
                    # emit the previous batch's normalize+output block here,
                    # after this batch's first chunk has filled the PE queue
                    if n == 0 and pending_finish is not None:
                        pending_finish()
                        pending_finish = None

                def finish(b=b, scores=scores, zrow=zrow, acc=acc):
                    # ---- normalize: Z, attn out, ctx out ----
                    zsum = miscpool.tile([1, 1], f32, tag="zsum")
                    nc.vector.reduce_sum(zsum[:], zrow[:], axis=mybir.AxisListType.X)
                    sinv = miscpool.tile([1, 1], f32, tag="sinv")
                    nc.vector.reciprocal(sinv[:], zsum[:])
                    nc.vector.tensor_scalar_mul(scores[:], scores[:], sinv[:])
                    nc.sync.dma_start(out_attn[b : b + 1, :], scores[:])
                    sinv_bc = miscpool.tile([P, 1], f32, tag="sinvbc")
                    nc.gpsimd.partition_broadcast(sinv_bc[:], sinv[:])
                    acc_bf = miscpool.tile([P, KD], mdt, tag="accbf")
                    nc.vector.tensor_scalar_mul(acc_bf[:], acc[:], sinv_bc[:])
                    ptx = small_psum.tile([KD, P], f32, tag="sp", name="ptx")
                    nc.tensor.matmul(
                        ptx[:], lhsT=acc_bf[:], rhs=id_sb[:], start=True, stop=True
                    )
                    ctxrow = miscpool.tile([KD, P], f32, tag="ctxrow")
                    nc.vector.tensor_copy(ctxrow[:], ptx[:])
                    nc.sync.dma_start(
                        out_ctx[b].rearrange("(ko p) -> ko p", p=P), ctxrow[:]
                    )

                pending_finish = finish
            pending_finish()

    nc.compile()
    return nc


def _get_nc(mm_dt: str):
    if mm_dt not in _CACHE:
        _CACHE[mm_dt] = _build(mm_dt)
    return _CACHE[mm_dt]


def kernel(hidden, encoder_outputs, Wa, Ua, Va, _trace=False):
    mm_dt = MM_DT
    nc = _get_nc(mm_dt)

    wdt = ml_dtypes.bfloat16

    # host-side layout prep (sharding)
    encT_all = np.ascontiguousarray(encoder_outputs.transpose(1, 2, 0))  # [B, D, S]
    uaT_np = np.ascontiguousarray(Ua.T).astype(wdt)  # [D, H]
    waT_np = np.ascontiguousarray(Wa.T).astype(wdt)  # [H, H]
    va2_np = np.ascontiguousarray(Va[0].reshape(KH, P).T).astype(wdt)  # [P, KH]
    id_np = np.eye(P, dtype=wdt)

    in_maps = []
    for c in range(NCORES):
        b0 = c * BL
        in_maps.append(
            {
                "encT": encT_all[b0 : b0 + BL],
                "uaT": uaT_np,
                "waT": waT_np,
                "hidT": np.ascontiguousarray(hidden[b0 : b0 + BL].T).astype(wdt),
                "va2": va2_np,
                "id128": id_np,
            }
        )

    res = bass_utils.run_bass_kernel_spmd(
        nc, in_maps, core_ids=list(range(NCORES)), trace=_trace
    )

    ctx = np.concatenate([res.results[c]["out_ctx"] for c in range(NCORES)], axis=0)
    attn = np.concatenate([res.results[c]["out_attn"] for c in range(NCORES)], axis=0)
    out = (ctx.reshape(B, 1, D).astype(np.float32), attn.astype(np.float32))
    if _trace:
        return out, res
    return out


# revision 22
# speedup vs baseline: 1.3149x; 1.0154x over previous
"""Bahdanau attention on 8 Trainium2 NeuronCores.

Problem (full shapes): hidden [32,1024], encoder_outputs [2048,32,2048],
Wa [1024,1024], Ua [1024,2048], Va [1,1024].

reference:
    enc    = encoder_outputs.transpose(1,0,2)        # [B,S,2H]
    h_proj = hidden @ Wa.T                           # [B,H]
    e_proj = einsum('bsd,hd->bsh', enc, Ua)          # [B,S,H]
    energy = relu(h_proj[:,None,:] + e_proj)         # [B,S,H]
    scores = einsum('bsh,h->bs', energy, Va[0])      # [B,S]
    attn   = softmax(scores, axis=-1)                # [B,S]
    ctx    = einsum('bs,bsd->bd', attn, enc)[:,None] # [B,1,2H]
    return (ctx, attn)

Strategy: data-parallel over batch (4 batches/core).  Per batch the
dominant matmul is enc_b @ Ua.T, computed transposed as e_projT[h,s] so
the relu+h_proj bias is a per-partition scalar on the scalar engine and
the Va contraction is a K=H matmul on the tensor engine.  Softmax is
computed unnormalized (scores are O(+-10) so exp needs no max shift).
The context vector is folded flash-style per s-chunk out of the very
encoder tiles the e_proj matmul consumes, with fused multiply-reduce
(tensor_tensor_reduce) ops on VectorE against the exp-weight row
(partition-broadcast), rescaling by 1/Z at the end.  encoder_outputs is
therefore read from HBM exactly once.  Chunk loads/casts are emitted
one chunk ahead of the reduce chain so the in-order VectorE queue never
delays the tensor engine, and each batch's epilogue is deferred into
the next batch's instruction stream.
"""

import os
import sys

sys.path.insert(0, "/opt/trn_rl_repo")

import numpy as np
import ml_dtypes

import concourse.bacc as bacc
import concourse.tile as tile
import concourse.mybir as mybir
from concourse import bass_utils

B, S, H = 32, 2048, 1024
D = 2 * H
NCORES = 8
BL = B // NCORES  # batches per core
P = 128
NCH = 512          # matmul moving free-dim chunk (one PSUM bank)
KD = D // P        # 16 contraction tiles over D (e_proj)
KH = H // P        # 8 contraction tiles over H (scores)
NS = S // NCH      # 4 s-chunks

MM_DT = os.environ.get("BAH_MM_DT", "bf16")

_CACHE = {}


def _build(mm_dt: str):
    assert mm_dt == "bf16"
    f32 = mybir.dt.float32
    mdt = mybir.dt.bfloat16

    nc = bacc.Bacc("TRN2", target_bir_lowering=False, debug=False)
    encT = nc.declare_dram_parameter("encT", [BL, D, S], mdt, isOutput=False)
    uaT = nc.declare_dram_parameter("uaT", [D, H], mdt, isOutput=False)
    waT = nc.declare_dram_parameter("waT", [H, H], mdt, isOutput=False)
    hidT = nc.declare_dram_parameter("hidT", [H, BL], mdt, isOutput=False)
    va2 = nc.declare_dram_parameter("va2", [P, KH], mdt, isOutput=False)
    id128 = nc.declare_dram_parameter("id128", [P, P], mdt, isOutput=False)
    out_ctx = nc.declare_dram_parameter("out_ctx", [BL, D], f32, isOutput=True)
    out_attn = nc.declare_dram_parameter("out_attn", [BL, S], f32, isOutput=True)

    AF = mybir.ActivationFunctionType

    with tile.TileContext(nc) as tc:
        with (
            tc.tile_pool(name="weights", bufs=1) as wpool,
            tc.tile_pool(name="wa", bufs=6) as wapool,
            tc.tile_pool(name="etm", bufs=6) as etmpool,
            tc.tile_pool(name="en", bufs=3) as enpool,
            tc.tile_pool(name="sm", bufs=2) as smpool,
            tc.tile_pool(name="misc", bufs=2) as miscpool,
            tc.tile_pool(name="pe", bufs=3, space="PSUM") as pe_psum,
            tc.tile_pool(name="small", bufs=3, space="PSUM") as small_psum,
        ):
            def load_chunk(b, n, pieces=2):
                # DMA one [D, NCH] slab of (host-pre-bf16) encT; more pieces
                # -> the first matmul can start sooner (used for chunk 0)
                ssl = slice(n * NCH, (n + 1) * NCH)
                etm = etmpool.tile([P, KD, NCH], mdt, tag="etm", name="etm")
                kq = KD // pieces
                for h in range(pieces):
                    nc.sync.dma_start(
                        etm[:, h * kq : (h + 1) * kq, :],
                        encT[b, h * kq * P : (h + 1) * kq * P, ssl].rearrange(
                            "(ko p) s -> p ko s", p=P
                        ),
                    )
                return etm

            # first chunk's loads go out before the weight DMAs so the
            # tensor engine can start as early as possible
            etm_cur = load_chunk(0, 0, pieces=4)

            ua_sb = wpool.tile([P, KD, H], mdt)
            uaT_r = uaT[:].rearrange("(ko p) h -> p ko h", p=P)
            for k in range(KD):
                nc.sync.dma_start(ua_sb[:, k : k + 1, :], uaT_r[:, k : k + 1, :])
            va_sb = wpool.tile([P, KH], mdt)
            nc.sync.dma_start(va_sb[:], va2[:])
            id_sb = wpool.tile([P, P], mdt)
            nc.sync.dma_start(id_sb[:], id128[:])
            hid_sb = wpool.tile([P, KH, BL], mdt)
            nc.sync.dma_start(hid_sb[:], hidT[:].rearrange("(ko p) b -> p ko b", p=P))
            hp_sb = wpool.tile([P, KH, BL], f32)

            def emit_hp(m):
                # hp[h, b] = sum_k Wa.T[k, h] * hidden.T[k, b] for h-tile m
                php = small_psum.tile([P, BL], f32, tag="sp", name="php")
                for k in range(KH):
                    wa_t = wapool.tile([P, P], mdt, tag="wa")
                    nc.sync.dma_start(
                        wa_t[:], waT[k * P : (k + 1) * P, m * P : (m + 1) * P]
                    )
                    nc.tensor.matmul(
                        php[:],
                        lhsT=wa_t[:],
                        rhs=hid_sb[:, k, :],
                        start=(k == 0),
                        stop=(k == KH - 1),
                    )
                nc.vector.tensor_copy(hp_sb[:, m, :], php[:])

            pending_finish = None
            for b in range(BL):
                scores = smpool.tile([1, S], f32, tag="scores")
                zrow = miscpool.tile([1, NS], f32, tag="zrow")
                acc = miscpool.tile([P, KD], f32, tag="acc")
                for n in range(NS):
                    ssl = slice(n * NCH, (n + 1) * NCH)
                    etm = etm_cur
                    # prefetch the next chunk ahead of this chunk's reduce
                    # ops so the VectorE FIFO can't stall the tensor engine
                    if n + 1 < NS:
                        etm_cur = load_chunk(b, n + 1)
                    elif b + 1 < BL:
                        etm_cur = load_chunk(b + 1, 0)

                    scp = small_psum.tile([1, NCH], f32, tag="sp", name="scp")
                    for m in range(KH):
                        ps = pe_psum.tile([P, NCH], f32, tag="pe")
                        for k in range(KD):
                            nc.tensor.matmul(
                                ps[:],
                                lhsT=ua_sb[:, k, m * P : (m + 1) * P],
                                rhs=etm[:, k, :],
                                start=(k == 0),
                                stop=(k == KD - 1),
                            )
                        if b == 0 and n == 0:
                            emit_hp(m)
                        en = enpool.tile([P, NCH], mdt, tag="en")
                        nc.scalar.activation(
                            en[:], ps[:], AF.Relu, bias=hp_sb[:, m, b : b + 1]
                        )
                        nc.tensor.matmul(
                            scp[:],
                            lhsT=va_sb[:, m : m + 1],
                            rhs=en[:],
                            start=(m == 0),
                            stop=(m == KH - 1),
                        )
                    # exp (unnormalized softmax numerator) + running Z
                    nc.vector.tensor_copy(scores[:, ssl], scp[:])
                    nc.scalar.activation(
                        scores[:, ssl],
                        scores[:, ssl],
                        AF.Exp,
                        accum_out=zrow[:, n : n + 1],
                    )
                    exp_m = smpool.tile([1, NCH], mdt, tag="expm")
                    nc.vector.tensor_copy(exp_m[:], scores[:, ssl])
                    w_bc = miscpool.tile([P, NCH], mdt, tag="wbc")
                    nc.gpsimd.partition_broadcast(w_bc[:], exp_m[:])
                    # fused multiply+reduce folds this s-chunk's context
                    # contribution out of the encoder tiles in SBUF:
                    # acc[p, k] += sum_s etm[p, k, s] * exp[s]
                    rtmp = miscpool.tile([P, KD], f32, tag="rtmp", name="rtmp")
                    for k in range(KD):
                        junk = miscpool.tile([P, NCH], mdt, tag="junk", name="junk")
                        nc.vector.scalar_tensor_tensor(
                            out=junk[:],
                            in0=etm[:, k, :],
                            scalar=1.0,
                            in1=w_bc[:],
                            op0=mybir.AluOpType.mult,
                            op1=mybir.AluOpType.mult,
                            accum_out=rtmp[:, k : k + 1],
                        )
                    if n == 0:
                        nc.vector.tensor_copy(acc[:], rtmp[:])
                    else:
                        nc.vector.tensor_add(acc[:], acc[:], rtmp[:])
                    # emit the previous batch's normalize+output block here,
                    # after this batch's first chunk has filled the PE queue
                    if n == 0 and pending_finish is not None:
                        pending_finish()
                        pending_finish = None

                def finish(b=b, scores=scores, zrow=zrow, acc=acc):
                    # ---- normalize: Z, attn out, ctx out ----
                    zsum = miscpool.tile([1, 1], f32, tag="zsum")
                    nc.vector.reduce_sum(zsum[:], zrow[:], axis=mybir.AxisListType.X)
                    sinv = miscpool.tile([1, 1], f32, tag="sinv")
                    nc.vector.reciprocal(sinv[:], zsum[:])
                    nc.vector.tensor_scalar_mul(scores[:], scores[:], sinv[:])
                    nc.sync.dma_start(out_attn[b : b + 1, :], scores[:])
                    sinv_bc = miscpool.tile([P, 1], f32, tag="sinvbc")
                    nc.gpsimd.partition_broadcast(sinv_bc[:], sinv[:])
                    acc_bf = miscpool.tile([P, KD], mdt, tag="accbf")
                    nc.vector.tensor_scalar_mul(acc_bf[:], acc[:], sinv_bc[:])
                    ptx = small_psum.tile([KD, P], f32, tag="sp", name="ptx")
                    nc.tensor.matmul(
                        ptx[:], lhsT=acc_bf[:], rhs=id_sb[:], start=True, stop=True
                    )
                    ctxrow = miscpool.tile([KD, P], f32, tag="ctxrow")
                    nc.vector.tensor_copy(ctxrow[:], ptx[:])
                    nc.sync.dma_start(
                        out_ctx[b].rearrange("(ko p) -> ko p", p=P), ctxrow[:]
                    )

                pending_finish = finish
            pending_finish()

    nc.compile()
    return nc


def _get_nc(mm_dt: str):
    if mm_dt not in _CACHE:
        _CACHE[mm_dt] = _build(mm_dt)
    return _CACHE[mm_dt]


def kernel(hidden, encoder_outputs, Wa, Ua, Va, _trace=False):
    mm_dt = MM_DT
    nc = _get_nc(mm_dt)

    wdt = ml_dtypes.bfloat16

    # host-side layout prep (sharding); bf16 is the matmul compute dtype,
    # so cast during the host transpose rather than on-device
    encT_bf = np.ascontiguousarray(encoder_outputs.transpose(1, 2, 0)).astype(
        wdt
    )  # [B, D, S]
    uaT_np = np.ascontiguousarray(Ua.T).astype(wdt)  # [D, H]
    waT_np = np.ascontiguousarray(Wa.T).astype(wdt)  # [H, H]
    va2_np = np.ascontiguousarray(Va[0].reshape(KH, P).T).astype(wdt)  # [P, KH]
    id_np = np.eye(P, dtype=wdt)

    in_maps = []
    for c in range(NCORES):
        b0 = c * BL
        in_maps.append(
            {
                "encT": encT_bf[b0 : b0 + BL],
                "uaT": uaT_np,
                "waT": waT_np,
                "hidT": np.ascontiguousarray(hidden[b0 : b0 + BL].T).astype(wdt),
                "va2": va2_np,
                "id128": id_np,
            }
        )

    res = bass_utils.run_bass_kernel_spmd(
        nc, in_maps, core_ids=list(range(NCORES)), trace=_trace
    )

    ctx = np.concatenate([res.results[c]["out_ctx"] for c in range(NCORES)], axis=0)
    attn = np.concatenate([res.results[c]["out_attn"] for c in range(NCORES)], axis=0)
    out = (ctx.reshape(B, 1, D).astype(np.float32), attn.astype(np.float32))
    if _trace:
        return out, res
    return out


# revision 23
# speedup vs baseline: 1.3629x; 1.0365x over previous
"""Bahdanau attention on 8 Trainium2 NeuronCores.

Problem (full shapes): hidden [32,1024], encoder_outputs [2048,32,2048],
Wa [1024,1024], Ua [1024,2048], Va [1,1024].

reference:
    enc    = encoder_outputs.transpose(1,0,2)        # [B,S,2H]
    h_proj = hidden @ Wa.T                           # [B,H]
    e_proj = einsum('bsd,hd->bsh', enc, Ua)          # [B,S,H]
    energy = relu(h_proj[:,None,:] + e_proj)         # [B,S,H]
    scores = einsum('bsh,h->bs', energy, Va[0])      # [B,S]
    attn   = softmax(scores, axis=-1)                # [B,S]
    ctx    = einsum('bs,bsd->bd', attn, enc)[:,None] # [B,1,2H]
    return (ctx, attn)

Strategy: data-parallel over batch (4 batches/core).  Per batch the
dominant matmul is enc_b @ Ua.T, computed transposed as e_projT[h,s] so
the relu+h_proj bias is a per-partition scalar on the scalar engine and
the Va contraction is a K=H matmul on the tensor engine.  Softmax is
computed unnormalized (scores are O(+-10) so exp needs no max shift).
The context vector is folded flash-style per s-chunk out of the very
encoder tiles the e_proj matmul consumes, with fused multiply-reduce
(tensor_tensor_reduce) ops on VectorE against the exp-weight row
(partition-broadcast), rescaling by 1/Z at the end.  encoder_outputs is
therefore read from HBM exactly once.  Chunk loads/casts are emitted
one chunk ahead of the reduce chain so the in-order VectorE queue never
delays the tensor engine, and each batch's epilogue is deferred into
the next batch's instruction stream.
"""

import os
import sys

sys.path.insert(0, "/opt/trn_rl_repo")

import numpy as np
import ml_dtypes

import concourse.bacc as bacc
import concourse.tile as tile
import concourse.mybir as mybir
from concourse import bass_utils

B, S, H = 32, 2048, 1024
D = 2 * H
NCORES = 8
BL = B // NCORES  # batches per core
P = 128
NCH = 512          # matmul moving free-dim chunk (one PSUM bank)
KD = D // P        # 16 contraction tiles over D (e_proj)
KH = H // P        # 8 contraction tiles over H (scores)
NS = S // NCH      # 4 s-chunks

MM_DT = os.environ.get("BAH_MM_DT", "bf16")

_CACHE = {}


def _build(mm_dt: str):
    assert mm_dt == "bf16"
    f32 = mybir.dt.float32
    mdt = mybir.dt.bfloat16

    nc = bacc.Bacc("TRN2", target_bir_lowering=False, debug=False)
    encT = nc.declare_dram_parameter("encT", [BL, D, S], mdt, isOutput=False)
    uaT = nc.declare_dram_parameter("uaT", [D, H], mdt, isOutput=False)
    waT = nc.declare_dram_parameter("waT", [H, H], mdt, isOutput=False)
    hidT = nc.declare_dram_parameter("hidT", [H, BL], mdt, isOutput=False)
    va2 = nc.declare_dram_parameter("va2", [P, KH], mdt, isOutput=False)
    id128 = nc.declare_dram_parameter("id128", [P, P], mdt, isOutput=False)
    out_ctx = nc.declare_dram_parameter("out_ctx", [BL, D], f32, isOutput=True)
    out_attn = nc.declare_dram_parameter("out_attn", [BL, S], f32, isOutput=True)

    AF = mybir.ActivationFunctionType

    with tile.TileContext(nc) as tc:
        with (
            tc.tile_pool(name="weights", bufs=1) as wpool,
            tc.tile_pool(name="etm", bufs=6) as etmpool,
            tc.tile_pool(name="en", bufs=3) as enpool,
            tc.tile_pool(name="sm", bufs=2) as smpool,
            tc.tile_pool(name="misc", bufs=2) as miscpool,
            tc.tile_pool(name="pe", bufs=3, space="PSUM") as pe_psum,
            tc.tile_pool(name="small", bufs=3, space="PSUM") as small_psum,
        ):
            def load_chunk(b, n, pieces=2):
                # DMA one [D, NCH] slab of (host-pre-bf16) encT; more pieces
                # -> the first matmul can start sooner (used for chunk 0)
                ssl = slice(n * NCH, (n + 1) * NCH)
                etm = etmpool.tile([P, KD, NCH], mdt, tag="etm", name="etm")
                kq = KD // pieces
                for h in range(pieces):
                    nc.sync.dma_start(
                        etm[:, h * kq : (h + 1) * kq, :],
                        encT[b, h * kq * P : (h + 1) * kq * P, ssl].rearrange(
                            "(ko p) s -> p ko s", p=P
                        ),
                    )
                return etm

            # first chunk's loads go out before the weight DMAs so the
            # tensor engine can start as early as possible
            etm_cur = load_chunk(0, 0, pieces=4)

            ua_sb = wpool.tile([P, KD, H], mdt)
            uaT_r = uaT[:].rearrange("(ko p) h -> p ko h", p=P)
            nc.sync.dma_start(ua_sb[:, 0:1, :], uaT_r[:, 0:1, :])
            wa_sb = wpool.tile([P, KH, H], mdt)
            nc.sync.dma_start(wa_sb[:], waT[:].rearrange("(ko p) h -> p ko h", p=P))
            for k in range(1, KD):
                nc.sync.dma_start(ua_sb[:, k : k + 1, :], uaT_r[:, k : k + 1, :])
            va_sb = wpool.tile([P, KH], mdt)
            nc.sync.dma_start(va_sb[:], va2[:])
            id_sb = wpool.tile([P, P], mdt)
            nc.sync.dma_start(id_sb[:], id128[:])
            hid_sb = wpool.tile([P, KH, BL], mdt)
            nc.sync.dma_start(hid_sb[:], hidT[:].rearrange("(ko p) b -> p ko b", p=P))
            hp_sb = wpool.tile([P, KH, BL], f32)

            def emit_hp(m):
                # hp[h, b] = sum_k Wa.T[k, h] * hidden.T[k, b] for h-tile m
                php = small_psum.tile([P, BL], f32, tag="sp", name="php")
                for k in range(KH):
                    nc.tensor.matmul(
                        php[:],
                        lhsT=wa_sb[:, k, m * P : (m + 1) * P],
                        rhs=hid_sb[:, k, :],
                        start=(k == 0),
                        stop=(k == KH - 1),
                    )
                nc.vector.tensor_copy(hp_sb[:, m, :], php[:])

            pending_finish = None
            for b in range(BL):
                scores = smpool.tile([1, S], f32, tag="scores")
                zrow = miscpool.tile([1, NS], f32, tag="zrow")
                acc = miscpool.tile([P, KD], f32, tag="acc")
                for n in range(NS):
                    ssl = slice(n * NCH, (n + 1) * NCH)
                    etm = etm_cur
                    # prefetch the next chunk ahead of this chunk's reduce
                    # ops so the VectorE FIFO can't stall the tensor engine
                    if n + 1 < NS:
                        etm_cur = load_chunk(b, n + 1)
                    elif b + 1 < BL:
                        etm_cur = load_chunk(b + 1, 0)

                    scp = small_psum.tile([1, NCH], f32, tag="sp", name="scp")
                    for m in range(KH):
                        ps = pe_psum.tile([P, NCH], f32, tag="pe")
                        for k in range(KD):
                            nc.tensor.matmul(
                                ps[:],
                                lhsT=ua_sb[:, k, m * P : (m + 1) * P],
                                rhs=etm[:, k, :],
                                start=(k == 0),
                                stop=(k == KD - 1),
                            )
                        if b == 0 and n == 0:
                            emit_hp(m)
                        en = enpool.tile([P, NCH], mdt, tag="en")
                        nc.scalar.activation(
                            en[:], ps[:], AF.Relu, bias=hp_sb[:, m, b : b + 1]
                        )
                        nc.tensor.matmul(
                            scp[:],
                            lhsT=va_sb[:, m : m + 1],
                            rhs=en[:],
                            start=(m == 0),
                            stop=(m == KH - 1),
                        )
                    # exp (unnormalized softmax numerator) + running Z
                    nc.vector.tensor_copy(scores[:, ssl], scp[:])
                    nc.scalar.activation(
                        scores[:, ssl],
                        scores[:, ssl],
                        AF.Exp,
                        accum_out=zrow[:, n : n + 1],
                    )
                    exp_m = smpool.tile([1, NCH], mdt, tag="expm")
                    nc.vector.tensor_copy(exp_m[:], scores[:, ssl])
                    w_bc = miscpool.tile([P, NCH], mdt, tag="wbc")
                    nc.gpsimd.partition_broadcast(w_bc[:], exp_m[:])
                    # fused multiply+reduce folds this s-chunk's context
                    # contribution out of the encoder tiles in SBUF:
                    # acc[p, k] += sum_s etm[p, k, s] * exp[s]
                    rtmp = miscpool.tile([P, KD], f32, tag="rtmp", name="rtmp")
                    for k in range(KD):
                        junk = miscpool.tile([P, NCH], mdt, tag="junk", name="junk")
                        nc.vector.scalar_tensor_tensor(
                            out=junk[:],
                            in0=etm[:, k, :],
                            scalar=1.0,
                            in1=w_bc[:],
                            op0=mybir.AluOpType.mult,
                            op1=mybir.AluOpType.mult,
                            accum_out=rtmp[:, k : k + 1],
                        )
                    if n == 0:
                        nc.vector.tensor_copy(acc[:], rtmp[:])
                    else:
                        nc.vector.tensor_add(acc[:], acc[:], rtmp[:])
                    # emit the previous batch's normalize+output block here,
                    # after this batch's first chunk has filled the PE queue
                    if n == 0 and pending_finish is not None:
                        pending_finish()
                        pending_finish = None

                def finish(b=b, scores=scores, zrow=zrow, acc=acc):
                    # ---- normalize: Z, attn out, ctx out ----
                    zsum = miscpool.tile([1, 1], f32, tag="zsum")
                    nc.vector.reduce_sum(zsum[:], zrow[:], axis=mybir.AxisListType.X)
                    sinv = miscpool.tile([1, 1], f32, tag="sinv")
                    nc.vector.reciprocal(sinv[:], zsum[:])
                    nc.vector.tensor_scalar_mul(scores[:], scores[:], sinv[:])
                    nc.sync.dma_start(out_attn[b : b + 1, :], scores[:])
                    sinv_bc = miscpool.tile([P, 1], f32, tag="sinvbc")
                    nc.gpsimd.partition_broadcast(sinv_bc[:], sinv[:])
                    acc_bf = miscpool.tile([P, KD], mdt, tag="accbf")
                    nc.vector.tensor_scalar_mul(acc_bf[:], acc[:], sinv_bc[:])
                    ptx = small_psum.tile([KD, P], f32, tag="sp", name="ptx")
                    nc.tensor.matmul(
                        ptx[:], lhsT=acc_bf[:], rhs=id_sb[:], start=True, stop=True
                    )
                    ctxrow = miscpool.tile([KD, P], f32, tag="ctxrow")
                    nc.vector.tensor_copy(ctxrow[:], ptx[:])
                    nc.sync.dma_start(
                        out_ctx[b].rearrange("(ko p) -> ko p", p=P), ctxrow[:]
                    )

                pending_finish = finish
            pending_finish()

    nc.compile()
    return nc


def _get_nc(mm_dt: str):
    if mm_dt not in _CACHE:
        _CACHE[mm_dt] = _build(mm_dt)
    return _CACHE[mm_dt]


def kernel(hidden, encoder_outputs, Wa, Ua, Va, _trace=False):
    mm_dt = MM_DT
    nc = _get_nc(mm_dt)

    wdt = ml_dtypes.bfloat16

    # host-side layout prep (sharding); bf16 is the matmul compute dtype,
    # so cast during the host transpose rather than on-device
    encT_bf = np.ascontiguousarray(encoder_outputs.transpose(1, 2, 0)).astype(
        wdt
    )  # [B, D, S]
    uaT_np = np.ascontiguousarray(Ua.T).astype(wdt)  # [D, H]
    waT_np = np.ascontiguousarray(Wa.T).astype(wdt)  # [H, H]
    va2_np = np.ascontiguousarray(Va[0].reshape(KH, P).T).astype(wdt)  # [P, KH]
    id_np = np.eye(P, dtype=wdt)

    in_maps = []
    for c in range(NCORES):
        b0 = c * BL
        in_maps.append(
            {
                "encT": encT_bf[b0 : b0 + BL],
                "uaT": uaT_np,
                "waT": waT_np,
                "hidT": np.ascontiguousarray(hidden[b0 : b0 + BL].T).astype(wdt),
                "va2": va2_np,
                "id128": id_np,
            }
        )

    res = bass_utils.run_bass_kernel_spmd(
        nc, in_maps, core_ids=list(range(NCORES)), trace=_trace
    )

    ctx = np.concatenate([res.results[c]["out_ctx"] for c in range(NCORES)], axis=0)
    attn = np.concatenate([res.results[c]["out_attn"] for c in range(NCORES)], axis=0)
    out = (ctx.reshape(B, 1, D).astype(np.float32), attn.astype(np.float32))
    if _trace:
        return out, res
    return out
